# revision 12
# baseline (speedup 1.0000x reference)
"""Trainium2 Bass kernel for nn_CHARM_40200893891073.

Reference math: the Conv1d branch is dead code — the output is
    remap = exp(rowsum(emb) [:,None] * colsum(emb) [None,:]) / D
broadcast over the batch dim:  out[b, c, d] = remap[c, d]  for all b.

Strategy (data-parallel over batch, 8 cores):
  Each core computes remap [64, 256] on-chip from the replicated
  emb_weight and writes its [64, 64, 256] batch shard (4 MiB) to DRAM
  with a single broadcast-source DMA.  Per-core HBM traffic is just the
  output write, which is the memory roofline for this problem.

On-chip pipeline (raw bass; Tile's tail drain doesn't compile on this
walrus build):
  1. One DMA: emb [64,256] -> SBUF partitions 0-63 and 64-127
     (zero-stride DRAM source repeat).
  2. DVE: rowsum over the free axis -> rs [128, 1]; bf16 cast of emb
     for the PE.
  3. PE:  ones[64,128]^T @ emb_bf16 -> psum [128, 256] = colsum
     replicated into every partition (engines cannot
     partition-broadcast reads, so the matmul does the replication).
  4. ACT: remap[p, d] = Exp(psum[p, d] * rs[p] - ln(D))  (per-partition
     scale AP, input straight from PSUM).  The Exp PWP table is warmed
     by a dummy activation at kernel start so the table DMA overlaps
     the input phase.
  5. One DMA: remap [128, 256] with the zero-stride repeat dim
     OUTERMOST -> the full [64, 64, 256] shard (partition
     p = (b%2)*64 + c).  Repeat-outer descriptor order makes each
     SDMA engine's consecutive descriptors hit consecutive DRAM
     addresses (4 KB runs) for packet aggregation.
"""

import contextlib
import numpy as np

B, CH, L, D = 512, 64, 1024, 256
NCORES = 8
BS = B // NCORES  # batches per core

_CACHE: dict = {}

SKIP_CONST_INIT = True
WARMUP_EXP = True
BF16_MATMUL = True
DMA_SPLIT = 1  # 1: single out-DMA on sync; 2: halves on sync + scalar HWDGE


@contextlib.contextmanager
def _const_init_skipped(bass_mod):
    """Suppress the const-AP memsets + all-engine barrier Bass.__init__
    emits (this kernel uses none of them; the barrier costs ~3.5 us)."""
    if not SKIP_CONST_INIT:
        yield
        return
    orig_barrier = bass_mod.Bass.all_engine_barrier
    orig_memset = bass_mod.BassGpSimd.memset
    bass_mod.Bass.all_engine_barrier = lambda self, *a, **k: None
    bass_mod.BassGpSimd.memset = lambda self, *a, **k: None
    try:
        yield
    finally:
        bass_mod.Bass.all_engine_barrier = orig_barrier
        bass_mod.BassGpSimd.memset = orig_memset


def _build_nc():
    import concourse.bass as bass
    import concourse.mybir as mybir

    with _const_init_skipped(bass):
        nc = bass.Bass()
    emb = nc.dram_tensor("emb_weight", [CH, D], mybir.dt.float32, kind="ExternalInput")
    out = nc.dram_tensor("out", [BS, CH, D], mybir.dt.float32, kind="ExternalOutput")

    ln_d = float(np.log(float(D)))
    mm_dt = mybir.dt.bfloat16 if BF16_MATMUL else mybir.dt.float32

    with (
        nc.sbuf_tensor([128, D], mybir.dt.float32) as emb_sb,
        nc.sbuf_tensor([64, D], mm_dt) as emb_mm,
        nc.sbuf_tensor([128, 1], mybir.dt.float32) as rs_sb,
        nc.sbuf_tensor([64, 128], mm_dt) as ones_sb,
        nc.sbuf_tensor([128, 1], mybir.dt.float32) as bias_sb,
        nc.sbuf_tensor([1, 1], mybir.dt.float32) as warm_sb,
        nc.sbuf_tensor([128, D], mybir.dt.float32) as remap_sb,
        nc.psum_tensor([128, D], mybir.dt.float32) as psum_cs,
        nc.semaphore("dma_in") as dma_in,
        nc.semaphore("s_ones") as s_ones,
        nc.semaphore("s_red") as s_red,
        nc.semaphore("s_cast") as s_cast,
        nc.semaphore("s_cs") as s_cs,
        nc.semaphore("s_act") as s_act,
        nc.semaphore("dma_out") as dma_out,
        nc.Block() as block,
    ):

        @block.sync
        def _(sync):
            # emb -> partitions 0-63 and 64-127 in one DMA (DRAM src repeat)
            sync.dma_start(
                out=emb_sb[:, :],
                in_=emb[:, :].unsqueeze(0).to_broadcast((2, CH, D)),
            ).then_inc(dma_in, 16)
            sync.wait_ge(s_act, 1)
            # out[b, c, d] with b = b2*2 + bl  <-  remap_sb[bl*64 + c, d]
            # dest dim (bl c) merges to stride 256, count 128.
            out_v = out.rearrange("(b2 bl) c d -> (bl c) b2 d", bl=2)
            nrep = BS // 2
            if DMA_SPLIT <= 1:
                sync.dma_start(
                    out=out_v,
                    in_=remap_sb[:, :].unsqueeze(1).to_broadcast((128, nrep, D)),
                ).then_inc(dma_out, 16)
                sync.wait_ge(dma_out, 16)
            else:
                h = nrep // 2
                sync.dma_start(
                    out=out_v[:, 0:h, :],
                    in_=remap_sb[:, :].unsqueeze(1).to_broadcast((128, h, D)),
                ).then_inc(dma_out, 16)
                sync.wait_ge(dma_out, 32)

        @block.vector
        def _(vector):
            vector.memset(ones_sb[:, :], 1.0).then_inc(s_ones, 1)
            vector.memset(bias_sb[:, :], -ln_d)
            vector.wait_ge(dma_in, 16)
            if BF16_MATMUL:
                vector.tensor_copy(out=emb_mm[:, :], in_=emb_sb[0:CH, :]).then_inc(
                    s_cast, 1
                )
            vector.reduce_sum(
                out=rs_sb[:, 0:1], in_=emb_sb[:, :], axis=mybir.AxisListType.X
            ).then_inc(s_red, 1)

        @block.tensor
        def _(tensor):
            tensor.wait_ge(s_ones, 1)
            if BF16_MATMUL:
                tensor.wait_ge(s_cast, 1)
                rhs = emb_mm[:, :]
            else:
                tensor.wait_ge(dma_in, 16)
                rhs = emb_sb[0:CH, :]
            # out[p, d] = sum_c emb[c, d] = colsum[d], for every partition p
            tensor.matmul(
                psum_cs[:, :],
                lhsT=ones_sb[:, :],
                rhs=rhs,
                start=True,
                stop=True,
            ).then_inc(s_cs, 1)

        @block.scalar
        def _(scalar):
            if WARMUP_EXP:
                # load the Exp PWP table while inputs stream in.
                # (zero warm_sb first: const APs are skipped, so Exp's bias
                # must be an AP, and garbage SBUF could NaN-notify)
                scalar.mul(warm_sb[0:1, 0:1], warm_sb[0:1, 0:1], 0.0)
                scalar.activation(
                    out=warm_sb[0:1, 0:1],
                    in_=warm_sb[0:1, 0:1],
                    func=mybir.ActivationFunctionType.Exp,
                    bias=warm_sb[0:1, 0:1],
                    scale=0.0,
                )
            scalar.wait_ge(s_red, 1)
            scalar.wait_ge(s_cs, 1)
            scalar.activation(
                out=remap_sb[:, :],
                in_=psum_cs[:, :],
                func=mybir.ActivationFunctionType.Exp,
                bias=bias_sb[:, 0:1],
                scale=rs_sb[:, 0:1],
            ).then_inc(s_act, 1)
            if DMA_SPLIT > 1:
                # second half of the output on the other HWDGE ring
                h = BS // 4
                out_v = out.rearrange("(b2 bl) c d -> (bl c) b2 d", bl=2)
                scalar.dma_start(
                    out=out_v[:, h : 2 * h, :],
                    in_=remap_sb[:, :].unsqueeze(1).to_broadcast((128, h, D)),
                ).then_inc(dma_out, 16)

    return nc


LAST_RESULTS = None


def kernel(**inputs) -> np.ndarray:
    global LAST_RESULTS
    from concourse.bass_utils import run_bass_kernel_spmd

    emb = np.ascontiguousarray(inputs["emb_weight"], dtype=np.float32)
    assert emb.shape == (CH, D)

    if "nc" not in _CACHE:
        _CACHE["nc"] = _build_nc()
    nc = _CACHE["nc"]

    in_maps = [{"emb_weight": emb} for _ in range(NCORES)]
    res = run_bass_kernel_spmd(nc, in_maps, core_ids=list(range(NCORES)))
    LAST_RESULTS = res
    out = np.concatenate([r["out"] for r in res.results], axis=0)
    assert out.shape == (B, CH, D)
    return np.ascontiguousarray(out, dtype=np.float32)


# revision 15
# speedup vs baseline: 1.1400x; 1.1400x over previous
"""Trainium2 Bass kernel for nn_CHARM_40200893891073.

Reference math: the Conv1d branch is dead code — the output is
    remap = exp(rowsum(emb) [:,None] * colsum(emb) [None,:]) / D
broadcast over the batch dim:  out[b, c, d] = remap[c, d]  for all b.

Strategy (data-parallel over batch, 8 cores):
  Each core computes remap [64, 256] on-chip from the replicated
  emb_weight and writes its [64, 64, 256] batch shard (4 MiB) to DRAM
  with a single broadcast-source DMA.  Per-core HBM traffic is just the
  output write, which is the memory roofline for this problem.

On-chip pipeline (raw bass; Tile's tail drain doesn't compile on this
walrus build):
  1. One DMA: emb [64,256] -> SBUF partitions 0-63 and 64-127
     (zero-stride DRAM source repeat).
  2. DVE: rowsum over the free axis -> rs [128, 1]; bf16 cast of emb
     for the PE.
  3. PE:  ones[64,128]^T @ emb_bf16 -> psum [128, 256] = colsum
     replicated into every partition (engines cannot
     partition-broadcast reads, so the matmul does the replication).
  4. ACT: remap[p, d] = Exp(psum[p, d] * rs[p] - ln(D))  (per-partition
     scale AP, input straight from PSUM).  The Exp PWP table is warmed
     by a dummy activation at kernel start so the table DMA overlaps
     the input phase.
  5. One DMA: remap [128, 256] with the zero-stride repeat dim
     OUTERMOST -> the full [64, 64, 256] shard (partition
     p = (b%2)*64 + c).  Repeat-outer descriptor order makes each
     SDMA engine's consecutive descriptors hit consecutive DRAM
     addresses (4 KB runs) for packet aggregation.
"""

import contextlib
import numpy as np

B, CH, L, D = 512, 64, 1024, 256
NCORES = 8
BS = B // NCORES  # batches per core

_CACHE: dict = {}

SKIP_CONST_INIT = True
WARMUP_EXP = True
BF16_MATMUL = True
DMA_SPLIT = 2  # 1: single out-DMA on sync; 2: halves on sync + scalar HWDGE


@contextlib.contextmanager
def _const_init_skipped(bass_mod):
    """Suppress the const-AP memsets + all-engine barrier Bass.__init__
    emits (this kernel uses none of them; the barrier costs ~3.5 us)."""
    if not SKIP_CONST_INIT:
        yield
        return
    orig_barrier = bass_mod.Bass.all_engine_barrier
    orig_memset = bass_mod.BassGpSimd.memset
    bass_mod.Bass.all_engine_barrier = lambda self, *a, **k: None
    bass_mod.BassGpSimd.memset = lambda self, *a, **k: None
    try:
        yield
    finally:
        bass_mod.Bass.all_engine_barrier = orig_barrier
        bass_mod.BassGpSimd.memset = orig_memset


def _build_nc():
    import concourse.bass as bass
    import concourse.mybir as mybir

    with _const_init_skipped(bass):
        nc = bass.Bass()
    emb = nc.dram_tensor("emb_weight", [CH, D], mybir.dt.float32, kind="ExternalInput")
    out = nc.dram_tensor("out", [BS, CH, D], mybir.dt.float32, kind="ExternalOutput")

    ln_d = float(np.log(float(D)))
    mm_dt = mybir.dt.bfloat16 if BF16_MATMUL else mybir.dt.float32

    with (
        nc.sbuf_tensor([128, D], mybir.dt.float32) as emb_sb,
        nc.sbuf_tensor([64, D], mm_dt) as emb_mm,
        nc.sbuf_tensor([128, 1], mybir.dt.float32) as rs_sb,
        nc.sbuf_tensor([64, 128], mm_dt) as ones_sb,
        nc.sbuf_tensor([128, 1], mybir.dt.float32) as bias_sb,
        nc.sbuf_tensor([1, 1], mybir.dt.float32) as warm_sb,
        nc.sbuf_tensor([128, D], mybir.dt.float32) as remap_sb,
        nc.psum_tensor([128, D], mybir.dt.float32) as psum_cs,
        nc.semaphore("dma_in") as dma_in,
        nc.semaphore("s_ones") as s_ones,
        nc.semaphore("s_red") as s_red,
        nc.semaphore("s_cast") as s_cast,
        nc.semaphore("s_cs") as s_cs,
        nc.semaphore("s_act") as s_act,
        nc.semaphore("dma_out") as dma_out,
        nc.Block() as block,
    ):

        @block.sync
        def _(sync):
            # emb -> partitions 0-63 and 64-127 (two plain DMAs; a single
            # zero-stride DRAM-source DMA measured ~3 us slower to complete)
            sync.dma_start(out=emb_sb[0:CH, :], in_=emb[:, :]).then_inc(dma_in, 16)
            sync.dma_start(out=emb_sb[CH : 2 * CH, :], in_=emb[:, :]).then_inc(
                dma_in, 16
            )
            sync.wait_ge(s_act, 1)
            # out[b, c, d] with b = b2*2 + bl  <-  remap_sb[bl*64 + c, d]
            # dest dim (bl c) merges to stride 256, count 128.
            out_v = out.rearrange("(b2 bl) c d -> (bl c) b2 d", bl=2)
            nrep = BS // 2
            if DMA_SPLIT <= 1:
                sync.dma_start(
                    out=out_v,
                    in_=remap_sb[:, :].unsqueeze(1).to_broadcast((128, nrep, D)),
                ).then_inc(dma_out, 16)
                sync.wait_ge(dma_out, 16)
            else:
                h = nrep // 2
                sync.dma_start(
                    out=out_v[:, 0:h, :],
                    in_=remap_sb[:, :].unsqueeze(1).to_broadcast((128, h, D)),
                ).then_inc(dma_out, 16)
                sync.wait_ge(dma_out, 32)

        @block.vector
        def _(vector):
            vector.memset(ones_sb[:, :], 1.0).then_inc(s_ones, 1)
            vector.memset(bias_sb[:, :], -ln_d)
            vector.wait_ge(dma_in, 16)
            if BF16_MATMUL:
                vector.tensor_copy(out=emb_mm[:, :], in_=emb_sb[0:CH, :]).then_inc(
                    s_cast, 1
                )
            vector.wait_ge(dma_in, 32)
            vector.reduce_sum(
                out=rs_sb[:, 0:1], in_=emb_sb[:, :], axis=mybir.AxisListType.X
            ).then_inc(s_red, 1)

        @block.tensor
        def _(tensor):
            tensor.wait_ge(s_ones, 1)
            if BF16_MATMUL:
                tensor.wait_ge(s_cast, 1)
                rhs = emb_mm[:, :]
            else:
                tensor.wait_ge(dma_in, 16)
                rhs = emb_sb[0:CH, :]
            # out[p, d] = sum_c emb[c, d] = colsum[d], for every partition p
            tensor.matmul(
                psum_cs[:, :],
                lhsT=ones_sb[:, :],
                rhs=rhs,
                start=True,
                stop=True,
            ).then_inc(s_cs, 1)

        @block.scalar
        def _(scalar):
            if WARMUP_EXP:
                # load the Exp PWP table while inputs stream in.
                # (zero warm_sb first: const APs are skipped, so Exp's bias
                # must be an AP, and garbage SBUF could NaN-notify)
                scalar.mul(warm_sb[0:1, 0:1], warm_sb[0:1, 0:1], 0.0)
                scalar.activation(
                    out=warm_sb[0:1, 0:1],
                    in_=warm_sb[0:1, 0:1],
                    func=mybir.ActivationFunctionType.Exp,
                    bias=warm_sb[0:1, 0:1],
                    scale=0.0,
                )
            scalar.wait_ge(s_red, 1)
            scalar.wait_ge(s_cs, 1)
            scalar.activation(
                out=remap_sb[:, :],
                in_=psum_cs[:, :],
                func=mybir.ActivationFunctionType.Exp,
                bias=bias_sb[:, 0:1],
                scale=rs_sb[:, 0:1],
            ).then_inc(s_act, 1)
            if DMA_SPLIT > 1:
                # second half of the output on the other HWDGE ring
                h = BS // 4
                out_v = out.rearrange("(b2 bl) c d -> (bl c) b2 d", bl=2)
                scalar.dma_start(
                    out=out_v[:, h : 2 * h, :],
                    in_=remap_sb[:, :].unsqueeze(1).to_broadcast((128, h, D)),
                ).then_inc(dma_out, 16)

    return nc


LAST_RESULTS = None


def kernel(**inputs) -> np.ndarray:
    global LAST_RESULTS
    from concourse.bass_utils import run_bass_kernel_spmd

    emb = np.ascontiguousarray(inputs["emb_weight"], dtype=np.float32)
    assert emb.shape == (CH, D)

    if "nc" not in _CACHE:
        _CACHE["nc"] = _build_nc()
    nc = _CACHE["nc"]

    in_maps = [{"emb_weight": emb} for _ in range(NCORES)]
    res = run_bass_kernel_spmd(nc, in_maps, core_ids=list(range(NCORES)))
    LAST_RESULTS = res
    out = np.concatenate([r["out"] for r in res.results], axis=0)
    assert out.shape == (B, CH, D)
    return np.ascontiguousarray(out, dtype=np.float32)
